# revision 9
# baseline (speedup 1.0000x reference)
"""Trainium2 Bass kernel for AceStep sliding-window GQA attention.

Problem: B=2, S=2048, H=2048, 16 Q heads / 4 KV heads, D=128, window +-256, fp32.

Sharding: 8 cores = (batch b in {0,1}) x (kv-group g in {0..3}).
Each core owns 4 Q heads + 1 KV head and computes a partial output
(wo restricted to its head group); host sums 4 partials per batch.

On-device layout is fully transposed ([dim, token]) so that:
  - QKV projections:  qT[d,s] = wqT[H,d].T @ hsT[H,s]          (PE matmul)
  - RoPE rotate_half: rot(q) = R @ q  (128x128 rotation matrix) (PE matmul)
  - RMSNorm sum over d and softmax denominator sum over k
    (partition-axis reductions) via ones-vector matmuls          (PE matmul)
  - scoresT[k,q] = kT[d,k].T @ qT[d,q]                          (PE matmul)
  - PV: outT[d,q] = v_kd[k,d].T @ probsT[k,q]                   (PE matmul)
  - O-proj: finalT[ho,s] = woT[dq,ho].T @ attnT[dq,s]           (PE matmul)
Softmax is computed without max-subtraction: RMS-normed q,k bound
|score| <= sqrt(128) ~ 11.3, so exp stays in fp32 range.
Sliding window exploited at block level: only ~6 of 16 k-tiles per q-tile.
Matmuls run as float32r (full PE rate at N>=256, near-fp32 precision).
"""

import os
import sys
from contextlib import ExitStack

import numpy as np

for _p in ("/opt/trn_rl_repo", "/root/.axon_site/_ro/trn_rl_repo"):
    if os.path.isdir(_p) and _p not in sys.path:
        sys.path.insert(0, _p)

import concourse.bass as bass
import concourse.bacc as bacc
import concourse.mybir as mybir
from concourse import tile
from concourse.alu_op_type import AluOpType

F32 = mybir.dt.float32
F32R = mybir.dt.float32r
ACT = mybir.ActivationFunctionType

# problem dims (hardcoded per spec)
B, S, H, NHQ, NKV, D, WIN = 2, 2048, 2048, 16, 4, 128, 256
EPS = 1e-6
HPC = NHQ // NKV          # 4 q heads per core
DQ = HPC * D              # 512
P = 128
KT = H // P               # 16 contraction tiles
SQ = 512                  # s-quarter width for projections
NSQ = S // SQ
QTW = 256                 # attention q-tile width
NQT = S // QTW
NKTILES = (QTW + 2 * WIN) // P   # 6 k-tiles per q-tile
N_CORES = 8

_CACHE = {}


def build_nc():
    nc = bacc.Bacc(None, target_bir_lowering=False, debug=False)

    hsT = nc.dram_tensor("hsT", [H, S], F32R, kind="ExternalInput")
    wq_t = nc.dram_tensor("wq_t", [H, DQ], F32R, kind="ExternalInput")
    wk_t = nc.dram_tensor("wk_t", [H, D], F32R, kind="ExternalInput")
    wv_t = nc.dram_tensor("wv_t", [H, D], F32R, kind="ExternalInput")
    wo_t = nc.dram_tensor("wo_t", [DQ, H], F32R, kind="ExternalInput")
    cos_t = nc.dram_tensor("cos_t", [D, S], F32, kind="ExternalInput")
    sin2_t = nc.dram_tensor("sin2_t", [D, S], F32, kind="ExternalInput")
    rot_t = nc.dram_tensor("rot_t", [D, D], F32R, kind="ExternalInput")
    ones_d = nc.dram_tensor("ones_d", [P, P], F32R, kind="ExternalInput")
    masks_d = nc.dram_tensor("masks", [NKTILES, P, QTW], F32, kind="ExternalInput")
    outT = nc.dram_tensor("outT", [H, S], F32, kind="ExternalOutput")

    with tile.TileContext(nc) as tc:
        es = ExitStack()
        top = es.enter_context(tc.tile_pool(name="top", bufs=1))

        # const APs used by nc.scalar.activation float biases
        zc = top.tile([P, 1], F32)
        nc.vector.memset(zc[:, :], 0.0)
        nc.const_aps.aps[(F32, 0.0)] = zc[:, :]
        bq = top.tile([P, 1], F32)
        nc.vector.memset(bq[:, :], float(D * EPS))
        nc.const_aps.aps[(F32, float(D * EPS))] = bq[:, :]
        bk = top.tile([P, 1], F32)
        nc.vector.memset(bk[:, :], float(EPS))
        nc.const_aps.aps[(F32, float(EPS))] = bk[:, :]

        ones_t = top.tile([P, P], F32R)
        nc.sync.dma_start(out=ones_t[:, :], in_=ones_d[:, :])
        ident = top.tile([P, P], F32)
        nc.vector.memset(ident[:, :], 1.0)
        nc.gpsimd.affine_select(
            out=ident[:, :], in_=ident[:, :], pattern=[[-1, P]],
            compare_op=AluOpType.is_equal, fill=0.0, base=0, channel_multiplier=1,
        )
        rot_sb = top.tile([D, D], F32R)
        nc.sync.dma_start(out=rot_sb[:, :], in_=rot_t[:, :])

        qT = [top.tile([P, S], F32R, name=f"qT{h}") for h in range(HPC)]
        kTt = top.tile([P, S], F32R, name="kTt")
        vkd = top.tile([P, S], F32R, name="vkd")  # s-tile t at [:, t*P:(t+1)*P], [s,d] layout
        attnT = [top.tile([P, S], F32R, name=f"attnT{h}") for h in range(HPC)]

        # ---------------- Phase 1: QKV projections + RMSNorm + RoPE ----------
        with tc.tile_pool(name="ph1", bufs=1) as ph1, \
             tc.tile_pool(name="ph1p", bufs=1, space="PSUM") as ph1p:
            wq_sb = ph1.tile([P, KT * DQ], F32R)
            wk_sb = ph1.tile([P, KT * D], F32R)
            wv_sb = ph1.tile([P, KT * D], F32R)
            for k in range(KT):
                nc.sync.dma_start(out=wq_sb[:, k * DQ:(k + 1) * DQ], in_=wq_t[k * P:(k + 1) * P, :])
                nc.sync.dma_start(out=wk_sb[:, k * D:(k + 1) * D], in_=wk_t[k * P:(k + 1) * P, :])
                nc.sync.dma_start(out=wv_sb[:, k * D:(k + 1) * D], in_=wv_t[k * P:(k + 1) * P, :])
            cos_sb = ph1.tile([D, S], F32)
            nc.sync.dma_start(out=cos_sb[:, :], in_=cos_t[:, :])
            sin2_sb = ph1.tile([D, S], F32)
            nc.sync.dma_start(out=sin2_sb[:, :], in_=sin2_t[:, :])

            for sq in range(NSQ):
                s0 = sq * SQ
                hst = []
                for k in range(KT):
                    t = ph1.tile([P, SQ], F32R, tag="hst", bufs=8)
                    nc.sync.dma_start(out=t[:, :], in_=hsT[k * P:(k + 1) * P, s0:s0 + SQ])
                    hst.append(t)

                accs = [ph1p.tile([P, SQ], F32, tag=f"acc{m}", bufs=1, name=f"acc{m}_{sq}")
                        for m in range(HPC + 2)]
                for k in range(KT):
                    st, sp = (k == 0), (k == KT - 1)
                    for m in range(HPC):
                        nc.tensor.matmul(
                            accs[m][:, :],
                            wq_sb[:, k * DQ + m * D: k * DQ + (m + 1) * D],
                            hst[k][:, :], start=st, stop=sp)
                    nc.tensor.matmul(accs[HPC][:, :], wk_sb[:, k * D:(k + 1) * D],
                                     hst[k][:, :], start=st, stop=sp)
                    nc.tensor.matmul(accs[HPC + 1][:, :], wv_sb[:, k * D:(k + 1) * D],
                                     hst[k][:, :], start=st, stop=sp)

                # q heads + k: RMSNorm (scale folded for q) + RoPE
                for m in range(HPC + 1):
                    raw = accs[m]
                    is_q = m < HPC
                    dst = qT[m][:, s0:s0 + SQ] if is_q else kTt[:, s0:s0 + SQ]
                    sqt = ph1.tile([P, SQ], F32R, tag="sqt", bufs=3)
                    nc.scalar.activation(sqt[:, :], raw[:, :], ACT.Square)
                    ssq = ph1p.tile([1, SQ], F32, tag="aux", bufs=2)
                    nc.tensor.matmul(ssq[:, :], ones_t[:, 0:1], sqt[:, :])
                    rms = ph1.tile([1, SQ], F32, tag="row", bufs=4)
                    if is_q:
                        # rsqrt(ssq/D + eps) * D^-1/2 == rsqrt(ssq + D*D*eps/D) -> sqrt(ssq + D*eps)
                        nc.scalar.activation(rms[:, :], ssq[:, :], ACT.Sqrt,
                                             bias=float(D * EPS), scale=1.0)
                    else:
                        nc.scalar.activation(rms[:, :], ssq[:, :], ACT.Sqrt,
                                             bias=float(EPS), scale=1.0 / D)
                    inv = ph1.tile([1, SQ], F32R, tag="rowr", bufs=4)
                    with nc.allow_low_precision(reason="f32r rounding for matmul feed"):
                        nc.vector.reciprocal(inv[:, :], rms[:, :])
                    invB = ph1p.tile([P, SQ], F32, tag="aux", bufs=2)
                    nc.tensor.matmul(invB[:, :], ones_t[0:1, :], inv[:, :])
                    t1 = ph1.tile([P, SQ], F32R, tag="t1", bufs=3)
                    nc.vector.tensor_mul(t1[:, :], raw[:, :], sin2_sb[:, s0:s0 + SQ])
                    rotp = ph1p.tile([P, SQ], F32, tag="aux", bufs=2)
                    nc.tensor.matmul(rotp[:, :], rot_sb[:, :], t1[:, :])
                    t2 = ph1.tile([P, SQ], F32, tag="tmp", bufs=6)
                    nc.vector.tensor_mul(t2[:, :], raw[:, :], cos_sb[:, s0:s0 + SQ])
                    t3 = ph1.tile([P, SQ], F32, tag="tmp", bufs=6)
                    nc.vector.tensor_add(t3[:, :], t2[:, :], rotp[:, :])
                    nc.vector.tensor_mul(dst, t3[:, :], invB[:, :])

                # v: to SBUF, then transpose to [s, d]
                vsb = ph1.tile([P, SQ], F32, tag="tmp", bufs=6)
                nc.scalar.copy(vsb[:, :], accs[HPC + 1][:, :])
                for j in range(SQ // P):
                    vt = ph1p.tile([P, P], F32, tag="aux", bufs=2)
                    nc.tensor.transpose(vt[:, :], vsb[:, j * P:(j + 1) * P], ident[:, :])
                    nc.vector.tensor_copy(vkd[:, s0 + j * P: s0 + (j + 1) * P], vt[:, :])

        # ---------------- Phase 2+3 -----------------------------------------
        with tc.tile_pool(name="late", bufs=1) as late:
            wo_sb = late.tile([P, HPC * H], F32R)
            for dqt in range(HPC):
                nc.sync.dma_start(out=wo_sb[:, dqt * H:(dqt + 1) * H],
                                  in_=wo_t[dqt * P:(dqt + 1) * P, :])

            # ------- attention -------
            with tc.tile_pool(name="att", bufs=1) as att, \
                 tc.tile_pool(name="attp", bufs=1, space="PSUM") as attp:
                masks_sb = att.tile([P, NKTILES * QTW], F32)
                for t in range(NKTILES):
                    nc.sync.dma_start(out=masks_sb[:, t * QTW:(t + 1) * QTW], in_=masks_d[t])

                for h in range(HPC):
                    for qi in range(NQT):
                        q0 = qi * QTW
                        tl = [t for t in range(NKTILES) if 0 <= q0 - WIN + t * P <= S - P]
                        probs = {}
                        for t in tl:
                            ks = q0 - WIN + t * P
                            scp = attp.tile([P, QTW], F32, tag="sc", bufs=2)
                            nc.tensor.matmul(scp[:, :], kTt[:, ks:ks + P],
                                             qT[h][:, q0:q0 + QTW])
                            pe = att.tile([P, QTW], F32R, tag="probs", bufs=14)
                            nc.scalar.activation(pe[:, :], scp[:, :], ACT.Exp)
                            if t in (0, 1, 4, 5):  # diagonal-boundary tiles need masking
                                pm = att.tile([P, QTW], F32R, tag="probs", bufs=14)
                                nc.vector.tensor_mul(pm[:, :], pe[:, :],
                                                     masks_sb[:, t * QTW:(t + 1) * QTW])
                                probs[t] = pm
                            else:
                                probs[t] = pe
                        den = attp.tile([1, QTW], F32, tag="den", bufs=2)
                        pv = attp.tile([P, QTW], F32, tag="pv", bufs=2)
                        for i, t in enumerate(tl):
                            ks = q0 - WIN + t * P
                            st, sp = (i == 0), (i == len(tl) - 1)
                            nc.tensor.matmul(den[:, :], ones_t[:, 0:1], probs[t][:, :],
                                             start=st, stop=sp)
                            nc.tensor.matmul(pv[:, :], vkd[:, ks:ks + P], probs[t][:, :],
                                             start=st, stop=sp)
                        inv = att.tile([1, QTW], F32R, tag="arow", bufs=4)
                        with nc.allow_low_precision(reason="f32r rounding for matmul feed"):
                            nc.vector.reciprocal(inv[:, :], den[:, :])
                        invB = attp.tile([P, QTW], F32, tag="ainvB", bufs=2)
                        nc.tensor.matmul(invB[:, :], ones_t[0:1, :], inv[:, :])
                        pvs = att.tile([P, QTW], F32, tag="pvs", bufs=3)
                        nc.scalar.copy(pvs[:, :], pv[:, :])
                        nc.vector.tensor_mul(attnT[h][:, q0:q0 + QTW], pvs[:, :], invB[:, :])

            # ------- output projection -------
            with tc.tile_pool(name="op", bufs=1) as op, \
                 tc.tile_pool(name="opp", bufs=1, space="PSUM") as opp:
                for ho in range(H // P):
                    for st4 in range(NSQ):
                        s0 = st4 * SQ
                        ops = opp.tile([P, SQ], F32, tag="o", bufs=3)
                        for dqt in range(HPC):
                            nc.tensor.matmul(
                                ops[:, :],
                                wo_sb[:, dqt * H + ho * P: dqt * H + (ho + 1) * P],
                                attnT[dqt][:, s0:s0 + SQ],
                                start=(dqt == 0), stop=(dqt == HPC - 1))
                        ob = op.tile([P, SQ], F32, tag="ob", bufs=4)
                        if (ho + st4) % 2 == 0:
                            nc.scalar.copy(ob[:, :], ops[:, :])
                        else:
                            nc.vector.tensor_copy(ob[:, :], ops[:, :])
                        nc.sync.dma_start(out=outT[ho * P:(ho + 1) * P, s0:s0 + SQ], in_=ob[:, :])
        es.close()
    nc.compile()
    return nc


def _host_prep(inputs):
    hs = np.ascontiguousarray(np.asarray(inputs["hidden_states"], dtype=np.float32))
    cos = np.asarray(inputs["cos"], dtype=np.float32)
    sin = np.asarray(inputs["sin"], dtype=np.float32)
    wq = np.asarray(inputs["wq"], dtype=np.float32)
    wk = np.asarray(inputs["wk"], dtype=np.float32)
    wv = np.asarray(inputs["wv"], dtype=np.float32)
    wo = np.asarray(inputs["wo"], dtype=np.float32)

    cosT = np.ascontiguousarray(cos.T)
    sin2 = np.concatenate([sin[:, D // 2:], sin[:, :D // 2]], axis=1)
    sin2T = np.ascontiguousarray(sin2.T)

    rot = np.zeros((D, D), dtype=np.float32)
    half = D // 2
    for d in range(half):
        rot[d, d + half] = -1.0
    for d in range(half, D):
        rot[d, d - half] = 1.0
    rotT = np.ascontiguousarray(rot.T)

    # multiplicative post-exp masks per relative k-tile offset
    masks = np.zeros((NKTILES, P, QTW), dtype=np.float32)
    i = np.arange(P)[:, None]
    j = np.arange(QTW)[None, :]
    for t in range(NKTILES):
        delta = -WIN + t * P
        masks[t] = (np.abs(delta + i - j) <= WIN).astype(np.float32)

    hsT = [np.ascontiguousarray(hs[b].T) for b in range(B)]
    in_maps = []
    for c in range(N_CORES):
        b, g = divmod(c, NKV)
        in_maps.append({
            "hsT": hsT[b],
            "wq_t": np.ascontiguousarray(wq[g * DQ:(g + 1) * DQ, :].T),
            "wk_t": np.ascontiguousarray(wk[g * D:(g + 1) * D, :].T),
            "wv_t": np.ascontiguousarray(wv[g * D:(g + 1) * D, :].T),
            "wo_t": np.ascontiguousarray(wo[:, g * DQ:(g + 1) * DQ].T),
            "cos_t": cosT,
            "sin2_t": sin2T,
            "rot_t": rotT,
            "ones_d": np.ones((P, P), dtype=np.float32),
            "masks": masks,
        })
    return in_maps


def kernel(**inputs):
    from concourse.bass_utils import run_bass_kernel_spmd
    if "nc" not in _CACHE:
        _CACHE["nc"] = build_nc()
    nc = _CACHE["nc"]
    in_maps = _host_prep(inputs)
    trace = bool(int(os.environ.get("BASS_TRACE_RUN", "0")))
    kw = {}
    td = os.environ.get("BASS_TRACE_DIR")
    if td:
        os.makedirs(td, exist_ok=True)
        kw["tmpdir"] = td
    res = run_bass_kernel_spmd(nc, in_maps, core_ids=list(range(N_CORES)), trace=trace, **kw)
    _CACHE["last_results"] = res
    out = np.empty((B, S, NHQ * D), dtype=np.float32)
    for b in range(B):
        acc = res.results[4 * b]["outT"].astype(np.float32, copy=True)
        for g in range(1, NKV):
            acc += res.results[4 * b + g]["outT"]
        out[b] = acc.T
    return out


if __name__ == "__main__":
    nc = build_nc()
    print("built OK; instructions:",
          sum(1 for _ in nc.m.functions[0].instructions)
          if hasattr(nc.m.functions[0], "instructions") else "?")


# revision 11
# speedup vs baseline: 1.1085x; 1.1085x over previous
"""Trainium2 Bass kernel for AceStep sliding-window GQA attention.

Problem: B=2, S=2048, H=2048, 16 Q heads / 4 KV heads, D=128, window +-256, fp32.

Sharding: 8 cores = (batch b in {0,1}) x (kv-group g in {0..3}).
Each core owns 4 Q heads + 1 KV head and computes a partial output
(wo restricted to its head group); host sums 4 partials per batch.

On-device layout is fully transposed ([dim, token]) so that:
  - QKV projections:  qT[d,s] = wqT[H,d].T @ hsT[H,s]          (PE matmul)
  - RoPE rotate_half: rot(q) = R @ q  (128x128 rotation matrix) (PE matmul)
  - RMSNorm sum over d and softmax denominator sum over k
    (partition-axis reductions) via ones-vector matmuls          (PE matmul)
  - scoresT[k,q] = kT[d,k].T @ qT[d,q]                          (PE matmul)
  - PV: outT[d,q] = v_kd[k,d].T @ probsT[k,q]                   (PE matmul)
  - O-proj: finalT[ho,s] = woT[dq,ho].T @ attnT[dq,s]           (PE matmul)
Softmax is computed without max-subtraction: RMS-normed q,k bound
|score| <= sqrt(128) ~ 11.3, so exp stays in fp32 range.
Sliding window exploited at block level: only ~6 of 16 k-tiles per q-tile.
Matmuls run as float32r (full PE rate at N>=256, near-fp32 precision).
"""

import os
import sys
from contextlib import ExitStack

import numpy as np

for _p in ("/opt/trn_rl_repo", "/root/.axon_site/_ro/trn_rl_repo"):
    if os.path.isdir(_p) and _p not in sys.path:
        sys.path.insert(0, _p)

import concourse.bass as bass
import concourse.bacc as bacc
import concourse.mybir as mybir
from concourse import tile
from concourse.alu_op_type import AluOpType

F32 = mybir.dt.float32
F32R = mybir.dt.float32r
ACT = mybir.ActivationFunctionType

# problem dims (hardcoded per spec)
B, S, H, NHQ, NKV, D, WIN = 2, 2048, 2048, 16, 4, 128, 256
EPS = 1e-6
HPC = NHQ // NKV          # 4 q heads per core
DQ = HPC * D              # 512
P = 128
KT = H // P               # 16 contraction tiles
SQ = 512                  # s-quarter width for projections
NSQ = S // SQ
QTW = 256                 # attention q-tile width
NQT = S // QTW
NKTILES = (QTW + 2 * WIN) // P   # 6 k-tiles per q-tile
N_CORES = 8

_CACHE = {}


def build_nc():
    nc = bacc.Bacc(None, target_bir_lowering=False, debug=False)

    hsT = nc.dram_tensor("hsT", [H, S], F32R, kind="ExternalInput")
    wq_t = nc.dram_tensor("wq_t", [H, DQ], F32R, kind="ExternalInput")
    wk_t = nc.dram_tensor("wk_t", [H, D], F32R, kind="ExternalInput")
    wv_t = nc.dram_tensor("wv_t", [H, D], F32R, kind="ExternalInput")
    wo_t = nc.dram_tensor("wo_t", [DQ, H], F32R, kind="ExternalInput")
    cos_t = nc.dram_tensor("cos_t", [D, S], F32, kind="ExternalInput")
    sin2_t = nc.dram_tensor("sin2_t", [D, S], F32, kind="ExternalInput")
    rot_t = nc.dram_tensor("rot_t", [D, D], F32R, kind="ExternalInput")
    ones_d = nc.dram_tensor("ones_d", [P, P], F32R, kind="ExternalInput")
    masks_d = nc.dram_tensor("masks", [NKTILES, P, QTW], F32, kind="ExternalInput")
    outT = nc.dram_tensor("outT", [H, S], F32, kind="ExternalOutput")

    with tile.TileContext(nc) as tc:
        es = ExitStack()
        top = es.enter_context(tc.tile_pool(name="top", bufs=1))

        # const APs used by nc.scalar.activation float biases
        zc = top.tile([P, 1], F32)
        nc.vector.memset(zc[:, :], 0.0)
        nc.const_aps.aps[(F32, 0.0)] = zc[:, :]
        bq = top.tile([P, 1], F32)
        nc.vector.memset(bq[:, :], float(D * EPS))
        nc.const_aps.aps[(F32, float(D * EPS))] = bq[:, :]
        bk = top.tile([P, 1], F32)
        nc.vector.memset(bk[:, :], float(EPS))
        nc.const_aps.aps[(F32, float(EPS))] = bk[:, :]

        ones_t = top.tile([P, P], F32R)
        nc.sync.dma_start(out=ones_t[:, :], in_=ones_d[:, :])
        ident = top.tile([P, P], F32)
        nc.vector.memset(ident[:, :], 1.0)
        nc.gpsimd.affine_select(
            out=ident[:, :], in_=ident[:, :], pattern=[[-1, P]],
            compare_op=AluOpType.is_equal, fill=0.0, base=0, channel_multiplier=1,
        )
        rot_sb = top.tile([D, D], F32R)
        nc.sync.dma_start(out=rot_sb[:, :], in_=rot_t[:, :])

        qT = [top.tile([P, S], F32R, name=f"qT{h}") for h in range(HPC)]
        kTt = top.tile([P, S], F32R, name="kTt")
        vkd = top.tile([P, S], F32R, name="vkd")  # s-tile t at [:, t*P:(t+1)*P], [s,d] layout
        attnT = [top.tile([P, S], F32R, name=f"attnT{h}") for h in range(HPC)]

        # ---------------- Phase 1: QKV projections + RMSNorm + RoPE ----------
        with tc.tile_pool(name="ph1", bufs=1) as ph1, \
             tc.tile_pool(name="ph1p", bufs=1, space="PSUM") as ph1p:
            wq_sb = ph1.tile([P, KT * DQ], F32R)
            wk_sb = ph1.tile([P, KT * D], F32R)
            wv_sb = ph1.tile([P, KT * D], F32R)
            for k in range(KT):
                nc.sync.dma_start(out=wq_sb[:, k * DQ:(k + 1) * DQ], in_=wq_t[k * P:(k + 1) * P, :])
                nc.sync.dma_start(out=wk_sb[:, k * D:(k + 1) * D], in_=wk_t[k * P:(k + 1) * P, :])
                nc.sync.dma_start(out=wv_sb[:, k * D:(k + 1) * D], in_=wv_t[k * P:(k + 1) * P, :])
            cos_sb = ph1.tile([D, S], F32)
            nc.sync.dma_start(out=cos_sb[:, :], in_=cos_t[:, :])
            sin2_sb = ph1.tile([D, S], F32)
            nc.sync.dma_start(out=sin2_sb[:, :], in_=sin2_t[:, :])

            for sq in range(NSQ):
                s0 = sq * SQ
                hst = []
                for k in range(KT):
                    t = ph1.tile([P, SQ], F32R, tag="hst", bufs=8)
                    nc.sync.dma_start(out=t[:, :], in_=hsT[k * P:(k + 1) * P, s0:s0 + SQ])
                    hst.append(t)

                accs = [ph1p.tile([P, SQ], F32, tag=f"acc{m}", bufs=1, name=f"acc{m}_{sq}")
                        for m in range(HPC + 2)]
                for k in range(KT):
                    st, sp = (k == 0), (k == KT - 1)
                    for m in range(HPC):
                        nc.tensor.matmul(
                            accs[m][:, :],
                            wq_sb[:, k * DQ + m * D: k * DQ + (m + 1) * D],
                            hst[k][:, :], start=st, stop=sp)
                    nc.tensor.matmul(accs[HPC][:, :], wk_sb[:, k * D:(k + 1) * D],
                                     hst[k][:, :], start=st, stop=sp)
                    nc.tensor.matmul(accs[HPC + 1][:, :], wv_sb[:, k * D:(k + 1) * D],
                                     hst[k][:, :], start=st, stop=sp)

                # q heads + k: RMSNorm (scale folded for q) + RoPE
                for m in range(HPC + 1):
                    raw = accs[m]
                    is_q = m < HPC
                    dst = qT[m][:, s0:s0 + SQ] if is_q else kTt[:, s0:s0 + SQ]
                    sqt = ph1.tile([P, SQ], F32R, tag="sqt", bufs=3)
                    nc.scalar.activation(sqt[:, :], raw[:, :], ACT.Square)
                    ssq = ph1p.tile([1, SQ], F32, tag="aux", bufs=2)
                    nc.tensor.matmul(ssq[:, :], ones_t[:, 0:1], sqt[:, :])
                    rms = ph1.tile([1, SQ], F32, tag="row", bufs=2)
                    if is_q:
                        # rsqrt(ssq/D + eps) * D^-1/2 == rsqrt(ssq + D*D*eps/D) -> sqrt(ssq + D*eps)
                        nc.scalar.activation(rms[:, :], ssq[:, :], ACT.Sqrt,
                                             bias=float(D * EPS), scale=1.0)
                    else:
                        nc.scalar.activation(rms[:, :], ssq[:, :], ACT.Sqrt,
                                             bias=float(EPS), scale=1.0 / D)
                    invf = ph1.tile([1, SQ], F32, tag="rowf", bufs=2)
                    nc.vector.reciprocal_approx_fast(out=invf[:, :], in_=rms[:, :])
                    inv = ph1.tile([1, SQ], F32R, tag="rowr", bufs=2)
                    nc.scalar.copy(inv[:, :], invf[:, :])
                    invB = ph1p.tile([P, SQ], F32, tag="aux", bufs=2)
                    nc.tensor.matmul(invB[:, :], ones_t[0:1, :], inv[:, :])
                    t1 = ph1.tile([P, SQ], F32R, tag="t1", bufs=3)
                    nc.vector.tensor_mul(t1[:, :], raw[:, :], sin2_sb[:, s0:s0 + SQ])
                    rotp = ph1p.tile([P, SQ], F32, tag="aux", bufs=2)
                    nc.tensor.matmul(rotp[:, :], rot_sb[:, :], t1[:, :])
                    t2 = ph1.tile([P, SQ], F32, tag="tmp", bufs=6)
                    nc.vector.tensor_mul(t2[:, :], raw[:, :], cos_sb[:, s0:s0 + SQ])
                    t3 = ph1.tile([P, SQ], F32, tag="tmp", bufs=6)
                    nc.vector.tensor_add(t3[:, :], t2[:, :], rotp[:, :])
                    nc.vector.tensor_mul(dst, t3[:, :], invB[:, :])

                # v: to SBUF, then transpose to [s, d]
                vsb = ph1.tile([P, SQ], F32, tag="tmp", bufs=6)
                nc.scalar.copy(vsb[:, :], accs[HPC + 1][:, :])
                for j in range(SQ // P):
                    vt = ph1p.tile([P, P], F32, tag="aux", bufs=2)
                    nc.tensor.transpose(vt[:, :], vsb[:, j * P:(j + 1) * P], ident[:, :])
                    nc.vector.tensor_copy(vkd[:, s0 + j * P: s0 + (j + 1) * P], vt[:, :])

        # ---------------- Phase 2+3 -----------------------------------------
        with tc.tile_pool(name="late", bufs=1) as late:
            wo_sb = late.tile([P, HPC * H], F32R)
            for dqt in range(HPC):
                nc.sync.dma_start(out=wo_sb[:, dqt * H:(dqt + 1) * H],
                                  in_=wo_t[dqt * P:(dqt + 1) * P, :])

            # ------- attention -------
            with tc.tile_pool(name="att", bufs=1) as att, \
                 tc.tile_pool(name="attp", bufs=1, space="PSUM") as attp:
                masks_sb = att.tile([P, NKTILES * QTW], F32)
                for t in range(NKTILES):
                    nc.sync.dma_start(out=masks_sb[:, t * QTW:(t + 1) * QTW], in_=masks_d[t])

                for h in range(HPC):
                    denrow = att.tile([1, S], F32, tag="denrow", bufs=2, name=f"denrow{h}")
                    pvs_list = []
                    for qi in range(NQT):
                        q0 = qi * QTW
                        tl = [t for t in range(NKTILES) if 0 <= q0 - WIN + t * P <= S - P]
                        probs = {}
                        for t in tl:
                            ks = q0 - WIN + t * P
                            scp = attp.tile([P, QTW], F32, tag="sc", bufs=3)
                            nc.tensor.matmul(scp[:, :], kTt[:, ks:ks + P],
                                             qT[h][:, q0:q0 + QTW])
                            pe = att.tile([P, QTW], F32R, tag="probs", bufs=14)
                            nc.scalar.activation(pe[:, :], scp[:, :], ACT.Exp)
                            if t in (0, 1, 4, 5):  # diagonal-boundary tiles need masking
                                pm = att.tile([P, QTW], F32R, tag="probs", bufs=14)
                                nc.vector.tensor_mul(pm[:, :], pe[:, :],
                                                     masks_sb[:, t * QTW:(t + 1) * QTW])
                                probs[t] = pm
                            else:
                                probs[t] = pe
                        den = attp.tile([1, QTW], F32, tag="den", bufs=2)
                        pv = attp.tile([P, QTW], F32, tag="pv", bufs=2)
                        for i, t in enumerate(tl):
                            ks = q0 - WIN + t * P
                            st, sp = (i == 0), (i == len(tl) - 1)
                            nc.tensor.matmul(den[:, :], ones_t[:, 0:1], probs[t][:, :],
                                             start=st, stop=sp)
                            nc.tensor.matmul(pv[:, :], vkd[:, ks:ks + P], probs[t][:, :],
                                             start=st, stop=sp)
                        pvs = att.tile([P, QTW], F32, tag="pvs", bufs=10)
                        nc.scalar.copy(pvs[:, :], pv[:, :])
                        nc.vector.tensor_copy(denrow[:, q0:q0 + QTW], den[:, :])
                        pvs_list.append(pvs)
                    invf = att.tile([1, S], F32, tag="invf", bufs=2)
                    nc.vector.reciprocal_approx_fast(out=invf[:, :], in_=denrow[:, :])
                    invr = att.tile([1, S], F32R, tag="invr", bufs=2)
                    nc.scalar.copy(invr[:, :], invf[:, :])
                    for qi in range(NQT):
                        q0 = qi * QTW
                        invB = attp.tile([P, QTW], F32, tag="ainvB", bufs=1)
                        nc.tensor.matmul(invB[:, :], ones_t[0:1, :], invr[:, q0:q0 + QTW])
                        nc.vector.tensor_mul(attnT[h][:, q0:q0 + QTW],
                                             pvs_list[qi][:, :], invB[:, :])

            # ------- output projection -------
            with tc.tile_pool(name="op", bufs=1) as op, \
                 tc.tile_pool(name="opp", bufs=1, space="PSUM") as opp:
                for ho in range(H // P):
                    for st4 in range(NSQ):
                        s0 = st4 * SQ
                        ops = opp.tile([P, SQ], F32, tag="o", bufs=3)
                        for dqt in range(HPC):
                            nc.tensor.matmul(
                                ops[:, :],
                                wo_sb[:, dqt * H + ho * P: dqt * H + (ho + 1) * P],
                                attnT[dqt][:, s0:s0 + SQ],
                                start=(dqt == 0), stop=(dqt == HPC - 1))
                        ob = op.tile([P, SQ], F32, tag="ob", bufs=4)
                        if (ho + st4) % 2 == 0:
                            nc.scalar.copy(ob[:, :], ops[:, :])
                        else:
                            nc.vector.tensor_copy(ob[:, :], ops[:, :])
                        nc.sync.dma_start(out=outT[ho * P:(ho + 1) * P, s0:s0 + SQ], in_=ob[:, :])
        es.close()
    nc.compile()
    return nc


def _host_prep(inputs):
    hs = np.ascontiguousarray(np.asarray(inputs["hidden_states"], dtype=np.float32))
    cos = np.asarray(inputs["cos"], dtype=np.float32)
    sin = np.asarray(inputs["sin"], dtype=np.float32)
    wq = np.asarray(inputs["wq"], dtype=np.float32)
    wk = np.asarray(inputs["wk"], dtype=np.float32)
    wv = np.asarray(inputs["wv"], dtype=np.float32)
    wo = np.asarray(inputs["wo"], dtype=np.float32)

    cosT = np.ascontiguousarray(cos.T)
    sin2 = np.concatenate([sin[:, D // 2:], sin[:, :D // 2]], axis=1)
    sin2T = np.ascontiguousarray(sin2.T)

    rot = np.zeros((D, D), dtype=np.float32)
    half = D // 2
    for d in range(half):
        rot[d, d + half] = -1.0
    for d in range(half, D):
        rot[d, d - half] = 1.0
    rotT = np.ascontiguousarray(rot.T)

    # multiplicative post-exp masks per relative k-tile offset
    masks = np.zeros((NKTILES, P, QTW), dtype=np.float32)
    i = np.arange(P)[:, None]
    j = np.arange(QTW)[None, :]
    for t in range(NKTILES):
        delta = -WIN + t * P
        masks[t] = (np.abs(delta + i - j) <= WIN).astype(np.float32)

    hsT = [np.ascontiguousarray(hs[b].T) for b in range(B)]
    in_maps = []
    for c in range(N_CORES):
        b, g = divmod(c, NKV)
        in_maps.append({
            "hsT": hsT[b],
            "wq_t": np.ascontiguousarray(wq[g * DQ:(g + 1) * DQ, :].T),
            "wk_t": np.ascontiguousarray(wk[g * D:(g + 1) * D, :].T),
            "wv_t": np.ascontiguousarray(wv[g * D:(g + 1) * D, :].T),
            "wo_t": np.ascontiguousarray(wo[:, g * DQ:(g + 1) * DQ].T),
            "cos_t": cosT,
            "sin2_t": sin2T,
            "rot_t": rotT,
            "ones_d": np.ones((P, P), dtype=np.float32),
            "masks": masks,
        })
    return in_maps


def kernel(**inputs):
    from concourse.bass_utils import run_bass_kernel_spmd
    if "nc" not in _CACHE:
        _CACHE["nc"] = build_nc()
    nc = _CACHE["nc"]
    in_maps = _host_prep(inputs)
    trace = bool(int(os.environ.get("BASS_TRACE_RUN", "0")))
    kw = {}
    td = os.environ.get("BASS_TRACE_DIR")
    if td:
        os.makedirs(td, exist_ok=True)
        kw["tmpdir"] = td
    res = run_bass_kernel_spmd(nc, in_maps, core_ids=list(range(N_CORES)), trace=trace, **kw)
    _CACHE["last_results"] = res
    out = np.empty((B, S, NHQ * D), dtype=np.float32)
    for b in range(B):
        acc = res.results[4 * b]["outT"].astype(np.float32, copy=True)
        for g in range(1, NKV):
            acc += res.results[4 * b + g]["outT"]
        out[b] = acc.T
    return out


if __name__ == "__main__":
    nc = build_nc()
    print("built OK; instructions:",
          sum(1 for _ in nc.m.functions[0].instructions)
          if hasattr(nc.m.functions[0], "instructions") else "?")
